# revision 1
# baseline (speedup 1.0000x reference)
"""Trainium2 Bass kernel for nn_GCN (3-layer GCN + center-pair readout).

Strategy (8 NeuronCores, SPMD):
  - Shard destination nodes across cores (12500 nodes/core). Every edge is
    assigned to the core owning its dst; scatter-add is local per core.
  - Layers 1,2: h = x @ W computed on the owning core's shard, AllGathered
    (fp16) into a Shared-DRAM full table; each core dma_gathers the rows for
    its edges (sorted by dst block, then by src for HBM locality), scales by
    the symmetric norm, and scatter-adds via one-hot matmuls accumulated in
    PSUM (transposed layout [feat, dst] so the next layer's matmul needs no
    transposes).
  - Layer 1 never materializes x = z_table[z]: it gathers rows of
    T1 = z_table @ W1 (computed on host) by z[src] directly.
  - Layer 3 only needs the 2 center nodes of each subgraph (250 dst/core),
    so its sweep covers just the ~1/50 of edges pointing at centers and
    accumulates into a single compact 256-column PSUM tile.
  - Readout (center node pairs, 2-layer MLP) is local per core; host
    concatenates the 8 [125,1] results.

Host-side prep is limited to index manipulation: edge sorting/padding,
degree/norm computation, int16 gather indices (dma_gather limit: the full
h table is gathered via 4 quarter views of 25000 rows each).
"""
import numpy as np
from contextlib import ExitStack

P = 128
H = 128
NCORES = 8
NQ = 4          # gather-table quarters (int16 index limit)
BG = 8          # dst blocks per PSUM group
GG = 48         # max chunks per dma_gather call
NPG = 100      # nodes per graph
GDT_NP = np.float16   # table/message dtype


# --------------------------------------------------------------------------
# host-side preprocessing
# --------------------------------------------------------------------------

def _build_structure(num_nodes, edge_index, z, maxz):
    N = int(num_nodes)
    NSH = N // NCORES
    QROWS = N // NQ
    NBLK = (NSH + P - 1) // P

    src = np.asarray(edge_index[0], dtype=np.int64)
    dst = np.asarray(edge_index[1], dtype=np.int64)
    loops = np.arange(N, dtype=np.int64)
    src = np.concatenate([src, loops])
    dst = np.concatenate([dst, loops])
    deg = np.bincount(dst, minlength=N).astype(np.float32)
    dinv = 1.0 / np.sqrt(np.maximum(deg, 1.0))
    norm = (dinv[src] * dinv[dst]).astype(np.float32)
    zsrc = np.asarray(z, dtype=np.int64)[src]

    core = dst // NSH
    q = src // QROWS
    b = (dst - core * NSH) // P
    dl = (dst - core * NSH) % P

    key = (core * NQ + q) * NBLK + b
    cnt = np.bincount(key, minlength=NCORES * NQ * NBLK).reshape(NCORES, NQ, NBLK)
    seg_chunks = np.maximum((cnt.max(axis=0) + P - 1) // P, 1)  # [NQ, NBLK]

    # sort by (core, q, block, src): src-ascending within each segment gives
    # the gather DMA engines HBM row locality
    order = np.lexsort((src, b, q, core))
    src_s, dl_s = src[order], dl[order]
    norm_s, zsrc_s = norm[order], zsrc[order]

    groups = [list(range(g, min(g + BG, NBLK))) for g in range(0, NBLK, BG)]

    NCHUNK = int(seg_chunks.sum())
    NSLOT = NCHUNK * P

    seg_off = np.zeros((NQ, NBLK), dtype=np.int64)
    cursor = 0
    chunk_blk = []   # block of each chunk
    call_plan = []   # (gi, q, chunk0, nchunks) -- split into <=GG sub-calls later
    chunk_bank = []  # (gi, bank-within-group-psum) of each chunk
    for gi, blocks in enumerate(groups):
        g0 = blocks[0]
        for qq in range(NQ):
            c0 = cursor
            for bb in blocks:
                nch = int(seg_chunks[qq, bb])
                seg_off[qq, bb] = cursor * P
                for ci in range(nch):
                    chunk_blk.append(bb)
                    chunk_bank.append((gi, (bb - g0) // 4))
                cursor += nch
            call_plan.append((gi, qq, c0, cursor - c0))
    assert cursor == NCHUNK
    # PSUM accumulation flags at zero-region (bank) granularity: start only on
    # the first chunk touching a (group, bank), stop only on the last.
    first_of = {}
    last_of = {}
    for ci, bkey in enumerate(chunk_bank):
        if bkey not in first_of:
            first_of[bkey] = ci
        last_of[bkey] = ci
    chunk_meta = [
        (chunk_blk[ci], first_of[chunk_bank[ci]] == ci,
         last_of[chunk_bank[ci]] == ci)
        for ci in range(NCHUNK)
    ]

    starts = np.zeros(NCORES * NQ * NBLK + 1, dtype=np.int64)
    np.cumsum(cnt.reshape(-1), out=starts[1:])

    # ---- layer-3 compact structure: only edges into center nodes ----------
    # centers of graph g are global nodes g*NPG, g*NPG+1; compact column
    # index = 2*(local graph) + (0|1); all 8 cores share one padded layout.
    is_c = (dst % NPG) < 2
    src3_all, dst3_all = src[is_c], dst[is_c]
    norm3_all = norm[is_c]
    core3 = dst3_all // NSH
    q3 = src3_all // QROWS
    cl3 = 2 * ((dst3_all % NSH) // NPG) + (dst3_all % NPG)  # [0, 250)
    cnt3 = np.bincount(core3 * NQ + q3, minlength=NCORES * NQ).reshape(NCORES, NQ)
    seg3_chunks = np.maximum((cnt3.max(axis=0) + P - 1) // P, 1)  # [NQ]
    NCHUNK3 = int(seg3_chunks.sum())
    NSLOT3 = NCHUNK3 * P
    order3 = np.lexsort((src3_all, q3, core3))
    src3_s, cl3_s, norm3_s = src3_all[order3], cl3[order3], norm3_all[order3]
    starts3 = np.zeros(NCORES * NQ + 1, dtype=np.int64)
    np.cumsum(cnt3.reshape(-1), out=starts3[1:])
    seg3_off = np.zeros(NQ, dtype=np.int64)
    c3 = 0
    for qq in range(NQ):
        seg3_off[qq] = c3 * P
        c3 += int(seg3_chunks[qq])

    MAXZP = ((maxz + P - 1) // P) * P
    # device call order: (group, quarter, block) — per-core valid counts let
    # the DMA engines skip the trailing -1 pads of every segment
    seg_order = [(qq, bb) for blocks in groups for qq in range(NQ)
                 for bb in blocks]
    per_core = []
    for c in range(NCORES):
        idxh = np.full(NSLOT, -1, dtype=np.int16)
        normw = np.zeros(NSLOT, dtype=GDT_NP)
        dlw = np.full(NSLOT, -1.0, dtype=GDT_NP)
        for qq in range(NQ):
            for bb in range(NBLK):
                k = (c * NQ + qq) * NBLK + bb
                s0, s1 = starts[k], starts[k + 1]
                n = s1 - s0
                o = seg_off[qq, bb]
                idxh[o:o + n] = (src_s[s0:s1] % QROWS).astype(np.int16)
                normw[o:o + n] = norm_s[s0:s1].astype(GDT_NP)
                dlw[o:o + n] = dl_s[s0:s1].astype(GDT_NP)
        cnts = np.array([cnt[c, qq, bb] for qq, bb in seg_order],
                        dtype=np.int32).reshape(1, -1)
        assert cnts.min() >= 1

        idx3 = np.zeros(NSLOT3, dtype=np.int16)
        norm3w = np.zeros(NSLOT3, dtype=GDT_NP)
        cl3w = np.full(NSLOT3, -1.0, dtype=GDT_NP)
        for qq in range(NQ):
            k = c * NQ + qq
            s0, s1 = starts3[k], starts3[k + 1]
            n = s1 - s0
            o = seg3_off[qq]
            idx3[o:o + n] = (src3_s[s0:s1] % QROWS).astype(np.int16)
            norm3w[o:o + n] = norm3_s[s0:s1].astype(GDT_NP)
            cl3w[o:o + n] = cl3_s[s0:s1].astype(GDT_NP)

        # layer-1 as a dense matmul: C1[z, local_dst] = sum of norms
        NSHP = NBLK * P
        ct1 = np.zeros((MAXZP, NSHP), dtype=np.float32)
        mc = core == c
        np.add.at(ct1, (zsrc[mc], dst[mc] - c * NSH), norm[mc])

        idxh0 = np.where(idxh < 0, np.int16(0), idxh)
        per_core.append({
            "idxh_neg": np.tile(idxh.reshape(-1, 16).T, (8, 1)).copy(),
            "idxh": np.tile(idxh0.reshape(-1, 16).T, (8, 1)).copy(),
            "normw": normw.reshape(NCHUNK, P).T.copy(),
            "dlw": dlw.reshape(NCHUNK, P).T.copy(),
            "cnts": cnts,
            "idx3": np.tile(idx3.reshape(-1, 16).T, (8, 1)).copy(),
            "norm3w": norm3w.reshape(NCHUNK3, P).T.copy(),
            "cl3w": cl3w.reshape(NCHUNK3, P).T.copy(),
            "CT1": ct1.astype(GDT_NP),
        })

    struct = {
        "N": N, "NSH": NSH, "QROWS": QROWS, "NBLK": NBLK,
        "NCHUNK": NCHUNK, "NSLOT": NSLOT,
        "seg_chunks": seg_chunks, "groups": groups,
        "chunk_meta": chunk_meta, "call_plan": call_plan,
        "seg_chunk_off": (seg_off // P).astype(np.int64),
        "NSEG": len(seg_order),
        "NCHUNK3": NCHUNK3, "NSLOT3": NSLOT3,
        "seg3_chunks": seg3_chunks, "MAXZP": MAXZP,
    }
    return struct, per_core


# --------------------------------------------------------------------------
# device kernel builder
# --------------------------------------------------------------------------

def _build_kernel(struct, num_graphs, maxz=1000):
    import concourse.bass as bass
    import concourse.tile as tile
    import concourse.mybir as mybir
    from concourse import bacc

    f32 = mybir.dt.float32
    f16 = mybir.dt.float16 if GDT_NP == np.float16 else mybir.dt.bfloat16
    i16 = mybir.dt.int16
    i32 = mybir.dt.int32
    RELU = mybir.ActivationFunctionType.Relu
    COPY = mybir.ActivationFunctionType.Identity

    N, NSH, QROWS = struct["N"], struct["NSH"], struct["QROWS"]
    NBLK, NCHUNK = struct["NBLK"], struct["NCHUNK"]
    NCHUNK3 = struct["NCHUNK3"]
    groups = struct["groups"]
    chunk_meta = struct["chunk_meta"]
    call_plan = struct["call_plan"]
    seg_chunks = struct["seg_chunks"]
    seg_chunk_off = struct["seg_chunk_off"]
    NSEG = struct["NSEG"]
    MAXSEGCH = int(np.asarray(seg_chunks).max())
    seg3_chunks = struct["seg3_chunks"]
    NSHP = NBLK * P                 # padded shard rows (12544)
    GSH = NSH // NPG                # graphs per core (125)
    W3C = 2 * GSH                   # compact layer-3 columns (250)

    import os as _os
    SEGCALLS = bool(int(_os.environ.get("GCN_SEGCALLS", "0")))
    NWBUF = 8 if SEGCALLS else 3

    nc = bacc.Bacc("TRN2", target_bir_lowering=False, debug=False,
                   num_devices=NCORES)

    MAXZP = struct["MAXZP"]
    NZC = MAXZP // P                # z chunks (8)

    # ---- I/O
    idxh_d = nc.dram_tensor("idxh", [P, struct["NSLOT"] // 16], i16, kind="ExternalInput")
    cnts_d = nc.dram_tensor("cnts", [1, NSEG], i32, kind="ExternalInput")
    ct1_d = nc.dram_tensor("CT1", [MAXZP, NBLK * P], f16, kind="ExternalInput")
    t1z_d = nc.dram_tensor("T1Z", [P, MAXZP], f16, kind="ExternalInput")
    norm_d = nc.dram_tensor("normw", [P, NCHUNK], f16, kind="ExternalInput")
    dl_d = nc.dram_tensor("dlw", [P, NCHUNK], f16, kind="ExternalInput")
    idx3_d = nc.dram_tensor("idx3", [P, struct["NSLOT3"] // 16], i16, kind="ExternalInput")
    norm3_d = nc.dram_tensor("norm3w", [P, NCHUNK3], f16, kind="ExternalInput")
    cl3_d = nc.dram_tensor("cl3w", [P, NCHUNK3], f16, kind="ExternalInput")
    W_d = [nc.dram_tensor(f"W{i}", [P, P], f32, kind="ExternalInput") for i in (1, 2, 3)]
    b_d = [nc.dram_tensor(f"b{i}", [P, 1], f32, kind="ExternalInput") for i in (1, 2, 3)]
    mw1_d = nc.dram_tensor("mw1", [P, P], f32, kind="ExternalInput")
    mw2_d = nc.dram_tensor("mw2", [P, 1], f32, kind="ExternalInput")
    mb1_d = nc.dram_tensor("mb1", [P, 1], f32, kind="ExternalInput")
    mb2_d = nc.dram_tensor("mb2", [1, 1], f32, kind="ExternalInput")
    y_d = nc.dram_tensor("y", [1, GSH], f32, kind="ExternalOutput")

    with tile.TileContext(nc) as tc, ExitStack() as ctx:
        dram = ctx.enter_context(tc.tile_pool(name="dram", bufs=1, space="DRAM"))
        const = ctx.enter_context(tc.tile_pool(name="const", bufs=1))
        work = ctx.enter_context(tc.tile_pool(name="work", bufs=4))
        msgp = ctx.enter_context(tc.tile_pool(name="msgp", bufs=NWBUF))
        ohp = ctx.enter_context(tc.tile_pool(name="ohp", bufs=NWBUF))
        stage_p = ctx.enter_context(tc.tile_pool(name="stagep", bufs=2))
        ps_sc = ctx.enter_context(tc.tile_pool(name="ps_sc", bufs=2, space="PSUM"))
        ps_mm = ctx.enter_context(tc.tile_pool(name="ps_mm", bufs=2, space="PSUM"))
        ps_l3 = ctx.enter_context(tc.tile_pool(name="ps_l3", bufs=1, space="PSUM"))

        hsh = dram.tile([NSHP, H], f16)

        # ---- constants
        iota_i = const.tile([P, 2 * P], i32)
        nc.gpsimd.iota(iota_i[:], pattern=[[1, 2 * P]], base=0, channel_multiplier=0)
        iota_h = const.tile([P, 2 * P], f16)
        nc.vector.tensor_copy(iota_h[:], iota_i[:])

        norm_t = const.tile([P, NCHUNK], f16)
        nc.sync.dma_start(norm_t[:], norm_d[:])
        dl_t = const.tile([P, NCHUNK], f16)
        nc.sync.dma_start(dl_t[:], dl_d[:])
        norm3_t = const.tile([P, NCHUNK3], f16)
        nc.sync.dma_start(norm3_t[:], norm3_d[:])
        cl3_t = const.tile([P, NCHUNK3], f16)
        nc.sync.dma_start(cl3_t[:], cl3_d[:])
        t1z_t = const.tile([P, NZC, P], f16)
        nc.sync.dma_start(t1z_t[:], t1z_d[:].rearrange("p (z f) -> p z f", f=P))
        W_t = []
        b_t = []
        for i in range(3):
            wf = const.tile([P, P], f32, name=f"wf{i}")
            nc.sync.dma_start(wf[:], W_d[i][:])
            w = const.tile([P, P], f16, name=f"w{i}")
            nc.vector.tensor_copy(w[:], wf[:])
            W_t.append(w)
            b = const.tile([P, 1], f32, name=f"bt{i}")
            nc.sync.dma_start(b[:], b_d[i][:])
            b_t.append(b)
        mw1_t = const.tile([P, P], f32)
        nc.sync.dma_start(mw1_t[:], mw1_d[:])
        mw2_t = const.tile([P, 1], f32)
        nc.sync.dma_start(mw2_t[:], mw2_d[:])
        mb1_t = const.tile([P, 1], f32)
        nc.sync.dma_start(mb1_t[:], mb1_d[:])
        mb2_t = const.tile([1, 1], f32)
        nc.sync.dma_start(mb2_t[:], mb2_d[:])

        xA = const.tile([P, NSHP], f16)
        xB = const.tile([P, NSHP], f16)

        # resident gather indices (removes per-call idx DMAs from the loop)
        idxh_t = const.tile([P, struct["NSLOT"] // 16], i16)
        nc.sync.dma_start(idxh_t[:], idxh_d[:])
        idx3_t = const.tile([P, struct["NSLOT3"] // 16], i16)
        nc.sync.dma_start(idx3_t[:], idx3_d[:])
        cnts_t = const.tile([1, NSEG], i32)
        nc.sync.dma_start(cnts_t[:], cnts_d[:])

        # msg pool buffers start as garbage SBUF; pad slots skipped by the
        # runtime-count gathers must hold FINITE stale data (0 * NaN = NaN)
        MSGCH = max(MAXSEGCH, int(np.asarray(seg3_chunks).max())) if SEGCALLS \
            else max(GG, int(np.asarray(seg3_chunks).max()))
        for _i in range(NWBUF):
            m0 = msgp.tile([P, MSGCH, H], f16, tag="msg")
            nc.vector.memset(m0[:], 0.0)

        # ---- scatter sweep helper
        import os as _os
        NOGATHER = bool(int(_os.environ.get("GCN_NOGATHER", "0")))
        NOMM = bool(int(_os.environ.get("GCN_NOMM", "0")))

        cnt_reg = nc.gpsimd.alloc_register("cnt_reg")

        def _sweep_body_seg(idx_sb, table_views, psg, gi, blocks):
            """One gather call per (quarter, block) segment; runtime per-core
            count skips the trailing pad descriptors."""
            g0 = blocks[0]
            si = 0
            for gj, bl in enumerate(groups):
                if gj < gi:
                    si += NQ * len(bl)
            for qq in range(NQ):
                for bb in blocks:
                    nch = int(seg_chunks[qq, bb])
                    cc0 = int(seg_chunk_off[qq, bb])
                    nidx = nch * P
                    msg = msgp.tile([P, nch, H], f16, tag="msg")
                    nc.gpsimd.reg_load(cnt_reg, cnts_t[0:1, si:si + 1])
                    nc.gpsimd.dma_gather(
                        msg[:], table_views[qq],
                        idx_sb[:, cc0 * 8:(cc0 + nch) * 8], nidx,
                        cnt_reg, H, single_packet=False)
                    nc.vector.tensor_tensor(
                        out=msg[:], in0=msg[:],
                        in1=norm_t[:, cc0:cc0 + nch][:, :, None]
                            .to_broadcast([P, nch, H]),
                        op=mybir.AluOpType.mult)
                    ohc = ohp.tile([P, nch, P], f16, tag="oh")
                    nc.vector.tensor_tensor(
                        out=ohc[:],
                        in0=iota_h[:, None, :P].to_broadcast([P, nch, P]),
                        in1=dl_t[:, cc0:cc0 + nch][:, :, None]
                            .to_broadcast([P, nch, P]),
                        op=mybir.AluOpType.is_equal)
                    col = (bb - g0) * P
                    for j in range(nch):
                        _, first, last = chunk_meta[cc0 + j]
                        nc.tensor.matmul(
                            psg[:, col:col + P], lhsT=msg[:, j, :],
                            rhs=ohc[:, j, :], start=first, stop=last)
                    si += 1

        def _sweep_body_big(idx_sb, table_views, psg, gi, blocks):
            """GG-chunk gather calls spanning a (group, quarter) run."""
            g0 = blocks[0]
            for qq in range(NQ):
                _, _, c0, nch = call_plan[gi * NQ + qq]
                s = 0
                while s < nch:
                    g = min(GG, nch - s)
                    cc0 = c0 + s
                    nidx = g * P
                    msg = msgp.tile([P, g, H], f16, tag="msg")
                    if NOGATHER:
                        nc.vector.memset(msg[:], 0.001)
                    else:
                        nc.gpsimd.dma_gather(
                            msg[:], table_views[qq],
                            idx_sb[:, cc0 * 8:(cc0 + g) * 8], nidx, nidx, H,
                            single_packet=False)
                    nc.vector.tensor_tensor(
                        out=msg[:], in0=msg[:],
                        in1=norm_t[:, cc0:cc0 + g][:, :, None]
                            .to_broadcast([P, g, H]),
                        op=mybir.AluOpType.mult)
                    ohc = ohp.tile([P, g, P], f16, tag="oh")
                    nc.vector.tensor_tensor(
                        out=ohc[:],
                        in0=iota_h[:, None, :P].to_broadcast([P, g, P]),
                        in1=dl_t[:, cc0:cc0 + g][:, :, None]
                            .to_broadcast([P, g, P]),
                        op=mybir.AluOpType.is_equal)
                    if not NOMM:
                        for j in range(g):
                            bb, first, last = chunk_meta[cc0 + j]
                            col = (bb - g0) * P
                            nc.tensor.matmul(
                                psg[:, col:col + P], lhsT=msg[:, j, :],
                                rhs=ohc[:, j, :], start=first, stop=last)
                    s += g

        def scatter_sweep(idx_sb, table_views, xout, bias_t, act):
            for gi, blocks in enumerate(groups):
                g0 = blocks[0]
                gw = len(blocks)
                psg = ps_sc.tile([P, gw * P], f32, tag="sc")
                if SEGCALLS:
                    _sweep_body_seg(idx_sb, table_views, psg, gi, blocks)
                else:
                    _sweep_body_big(idx_sb, table_views, psg, gi, blocks)
                # flush group: bias + (relu|copy), PSUM -> x buffer
                for bb in blocks:
                    bw = min(P, NSH - bb * P)
                    col = (bb - g0) * P
                    if NOMM:
                        nc.vector.memset(xout[:, bb * P:bb * P + bw], 0.0)
                    else:
                        nc.scalar.activation(
                            out=xout[:, bb * P:bb * P + bw],
                            in_=psg[:, col:col + bw],
                            func=act, bias=bias_t[:], scale=1.0)

        def sweep_l3(table_views, bias_t):
            """Compact layer-3 sweep: one [P, 256] PSUM over center columns."""
            ps3 = ps_l3.tile([P, 2 * P], f32, tag="l3")
            first_c = 0
            last_c = NCHUNK3 - 1
            cursor = 0
            for qq in range(NQ):
                nch = int(seg3_chunks[qq])
                s = 0
                while s < nch:
                    g = min(GG, nch - s)
                    cc0 = cursor + s
                    nidx = g * P
                    msg = msgp.tile([P, g, H], f16, tag="msg")
                    nc.gpsimd.dma_gather(
                        msg[:], table_views[qq],
                        idx3_t[:, cc0 * 8:(cc0 + g) * 8], nidx, nidx, H,
                        single_packet=False)
                    nc.vector.tensor_tensor(
                        out=msg[:],
                        in0=msg[:],
                        in1=norm3_t[:, cc0:cc0 + g][:, :, None]
                            .to_broadcast([P, g, H]),
                        op=mybir.AluOpType.mult)
                    oh = stage_p.tile([P, g, 2 * P], f16, tag="oh3")
                    nc.vector.tensor_tensor(
                        out=oh[:],
                        in0=iota_h[:, None, :].to_broadcast([P, g, 2 * P]),
                        in1=cl3_t[:, cc0:cc0 + g][:, :, None]
                            .to_broadcast([P, g, 2 * P]),
                        op=mybir.AluOpType.is_equal)
                    for j in range(g):
                        ci = cc0 + j
                        nc.tensor.matmul(
                            ps3[:], lhsT=msg[:, j, :], rhs=oh[:, j, :],
                            start=(ci == first_c), stop=(ci == last_c))
                    s += g
                cursor += nch
            x3c = const.tile([P, 2 * P], f32)
            nc.scalar.activation(out=x3c[:], in_=ps3[:], func=COPY,
                                 bias=bias_t[:], scale=1.0)
            return x3c

        # ---- layer 1 as dense matmul: xA = relu(T1^T @ C1 + b1)
        def l1_matmul():
            DC = 512
            for d0 in range(0, NSHP, DC):
                dw = min(DC, NSHP - d0)
                ps = ps_mm.tile([P, DC], f32, tag="mm")
                for zc in range(NZC):
                    ct = work.tile([P, DC], f16, tag="ct")
                    nc.sync.dma_start(ct[:, :dw],
                                      ct1_d[zc * P:(zc + 1) * P, d0:d0 + dw])
                    nc.tensor.matmul(ps[:, :dw], lhsT=t1z_t[:, zc, :],
                                     rhs=ct[:, :dw],
                                     start=(zc == 0), stop=(zc == NZC - 1))
                nc.scalar.activation(out=xA[:, d0:d0 + dw], in_=ps[:, :dw],
                                     func=RELU, bias=b_t[0][:], scale=1.0)

        # ---- h phase helper: hsh = x @ W -> AllGather -> hfull
        def h_phase(xin, w_t, hfull_t):
            for r0 in range(0, NBLK, 4):
                jn = min(4, NBLK - r0)
                st = stage_p.tile([P, 4, H], f16, tag="hst")
                for j in range(jn):
                    r = r0 + j
                    m = min(P, NSH - r * P)
                    ps = ps_mm.tile([P, P], f32, tag="mm")
                    nc.tensor.matmul(ps[:m, :], lhsT=xin[:, r * P:r * P + m],
                                     rhs=w_t[:], start=True, stop=True)
                    nc.vector.tensor_copy(st[:, j, :], ps[:, :])
                nc.sync.dma_start(
                    hsh[r0 * P:(r0 + jn) * P, :]
                        .rearrange("(j p) f -> p j f", p=P),
                    st[:, :jn, :])
            nc.gpsimd.collective_compute(
                "AllGather", mybir.AluOpType.bypass,
                replica_groups=[list(range(NCORES))],
                ins=[hsh[:NSH, :].opt()],
                outs=[hfull_t[:].opt()])

        # ---- layers
        STAGE = int(_os.environ.get("GCN_STAGE", "6"))
        REPS = int(_os.environ.get("GCN_REPS", "1"))
        for _rep in range(REPS):
            hfull = [dram.tile([N, H], f16, addr_space="Shared",
                               name=f"hfull{i}_{_rep}") for i in (2, 3)]
            if STAGE >= 1:
                l1_matmul()
            else:
                nc.vector.memset(xA[:], 0.0)
            if STAGE >= 2:
                h_phase(xA, W_t[1], hfull[0])
            if STAGE >= 3:
                h2_views = [hfull[0][qq * QROWS:(qq + 1) * QROWS, :] for qq in range(NQ)]
                scatter_sweep(idxh_t, h2_views, xB, b_t[1], RELU)
            else:
                nc.vector.memset(xB[:], 0.0)
            if STAGE >= 4:
                h_phase(xB, W_t[2], hfull[1])
            if STAGE >= 5:
                h3_views = [hfull[1][qq * QROWS:(qq + 1) * QROWS, :] for qq in range(NQ)]
                x3c = sweep_l3(h3_views, b_t[2])
            else:
                x3c = const.tile([P, 2 * P], f32, name=f"x3z_{_rep}")
                nc.vector.memset(x3c[:], 0.0)

            # ---- readout: p = x3[2g] * x3[2g+1]; y = relu(p@mw1+mb1)@mw2+mb2
            x3r = x3c[:, :W3C].rearrange("p (g r) -> p g r", r=2)
            pT = const.tile([P, GSH], f32, name=f"pT_{_rep}")
            nc.vector.tensor_tensor(
                out=pT[:], in0=x3r[:, :, 0], in1=x3r[:, :, 1],
                op=mybir.AluOpType.mult)
            hps = ps_mm.tile([P, GSH], f32, tag="mm")
            nc.tensor.matmul(hps[:], lhsT=mw1_t[:], rhs=pT[:], start=True, stop=True)
            hT = const.tile([P, GSH], f32, name=f"hT_{_rep}")
            nc.scalar.activation(out=hT[:], in_=hps[:], func=RELU,
                                 bias=mb1_t[:], scale=1.0)
            yps = ps_mm.tile([1, GSH], f32, tag="mm")
            nc.tensor.matmul(yps[:], lhsT=mw2_t[:], rhs=hT[:], start=True, stop=True)
            ysb = const.tile([1, GSH], f32, name=f"ysb_{_rep}")
            nc.scalar.activation(out=ysb[:], in_=yps[:], func=COPY,
                                 bias=mb2_t[:], scale=1.0)
            nc.sync.dma_start(y_d[:], ysb[:])

    nc.compile()
    return nc


# --------------------------------------------------------------------------
# entry point
# --------------------------------------------------------------------------

def _make_in_maps(inputs, per_core):
    import os as _os
    segcalls = bool(int(_os.environ.get("GCN_SEGCALLS", "0")))
    z_table = np.asarray(inputs["z_table"], np.float32)
    W1 = np.asarray(inputs["W1"], np.float32)
    maxz = z_table.shape[0]
    MAXZP = ((maxz + P - 1) // P) * P
    NZC = MAXZP // P
    t1 = np.zeros((MAXZP, H), np.float32)
    t1[:maxz] = z_table @ W1
    t1z = np.ascontiguousarray(
        t1.reshape(NZC, P, H).transpose(1, 0, 2).reshape(P, MAXZP)
    ).astype(GDT_NP)
    common = {
        "T1Z": t1z,
        "W1": W1, "W2": np.asarray(inputs["W2"], np.float32),
        "W3": np.asarray(inputs["W3"], np.float32),
        "b1": np.asarray(inputs["b1"], np.float32).reshape(P, 1),
        "b2": np.asarray(inputs["b2"], np.float32).reshape(P, 1),
        "b3": np.asarray(inputs["b3"], np.float32).reshape(P, 1),
        "mw1": np.asarray(inputs["mw1"], np.float32),
        "mw2": np.asarray(inputs["mw2"], np.float32).reshape(P, 1),
        "mb1": np.asarray(inputs["mb1"], np.float32).reshape(P, 1),
        "mb2": np.asarray(inputs["mb2"], np.float32).reshape(1, 1),
    }
    maps = []
    for c in range(NCORES):
        m = dict(common, **per_core[c])
        if segcalls:
            m["idxh"] = m["idxh_neg"]
        del m["idxh_neg"]
        maps.append(m)
    return maps


def kernel(num_nodes, z, edge_index, batch, num_graphs,
           z_table, W1, b1, W2, b2, W3, b3, mw1, mb1, mw2, mb2,
           _want_results=False):
    from concourse.bass_utils import run_bass_kernel_spmd

    num_nodes = int(num_nodes)
    num_graphs = int(num_graphs)
    z = np.asarray(z)
    edge_index = np.asarray(edge_index)

    struct, per_core = _build_structure(num_nodes, edge_index, z,
                                        np.asarray(z_table).shape[0])
    nc = _build_kernel(struct, num_graphs, maxz=np.asarray(z_table).shape[0])

    inputs = {"z_table": z_table, "W1": W1, "W2": W2, "W3": W3,
              "b1": b1, "b2": b2, "b3": b3, "mw1": mw1, "mw2": mw2,
              "mb1": mb1, "mb2": mb2}
    in_maps = _make_in_maps(inputs, per_core)

    res = run_bass_kernel_spmd(nc, in_maps, core_ids=list(range(NCORES)),
                               trace=bool(int(__import__("os").environ.get(
                                   "GCN_TRACE", "0"))))
    ys = [res.results[c]["y"].reshape(-1, 1) for c in range(NCORES)]
    out = np.concatenate(ys, 0).astype(np.float32)
    if _want_results:
        return out, res
    return out



# revision 2
# speedup vs baseline: 1.0766x; 1.0766x over previous
"""Trainium2 Bass kernel for nn_GCN (3-layer GCN + center-pair readout).

Strategy (8 NeuronCores, SPMD):
  - Shard destination nodes across cores (12500 nodes/core). Every edge is
    assigned to the core owning its dst; scatter-add is local per core.
  - norm factorization: norm(e) = dinv[src]*dinv[dst]. h tables are
    pre-scaled by dinv[row] when produced (free, fused into the PSUM->SBUF
    copy of the h phase), and the dst factor is applied once per dst column
    at PSUM flush time. This removes the per-edge message scaling pass
    entirely (the largest vector-engine cost in the naive scheme).
  - Layer 1 never materializes x = z_table[z]: x1 = relu(T1^T @ C1 + b1)
    with T1 = z_table @ W1 (host) and C1[z, dst] = sum of edge norms
    (host-built dense [1024, 12544] fp16 per core).
  - Layer 2's output x2 is only ever read at source nodes of edges into
    the per-graph center pairs (~33k of 100k nodes). The sweep therefore
    scatters only into that compact destination set (~3x less gather/
    one-hot/matmul work), and the second AllGather ships only the compact
    h3 rows.
  - Sweeps: AllGathered h table (fp16) in DRAM; each core dma_gathers the
    rows for its edges (sorted by dst block, then by src for locality),
    builds a one-hot (is_equal vs iota) and scatter-adds via matmuls
    accumulated in PSUM (transposed layout [feat, dst]).
  - Readout (center node pairs, 2-layer MLP) is local per core; host
    concatenates the 8 [125,1] results.
"""
import numpy as np
from contextlib import ExitStack

P = 128
H = 128
NCORES = 8
NQ = 4          # layer-2 gather-table quarters (int16 index limit)
NQ3 = 2         # layer-3 compact-table views
BG = 8          # dst blocks per PSUM group
GG = 48         # max chunks per dma_gather call
NPG = 100       # nodes per graph
GDT_NP = np.float16   # table/message dtype


# --------------------------------------------------------------------------
# host-side preprocessing
# --------------------------------------------------------------------------

def _pack_idx(idx):
    """[NSLOT] -> dma_gather index layout [128, NSLOT//16] (int16)."""
    return np.tile(idx.reshape(-1, 16).T, (8, 1)).copy()


def _chunk_layout(seg_chunks, groups, nq):
    """Group-major (group, quarter, block) chunk layout + PSUM accum flags."""
    nblk = seg_chunks.shape[1]
    seg_off = np.zeros((nq, nblk), dtype=np.int64)
    cursor = 0
    chunk_blk, chunk_bank, call_plan = [], [], []
    for gi, blocks in enumerate(groups):
        g0 = blocks[0]
        for qq in range(nq):
            c0 = cursor
            for bb in blocks:
                nch = int(seg_chunks[qq, bb])
                seg_off[qq, bb] = cursor * P
                for _ in range(nch):
                    chunk_blk.append(bb)
                    chunk_bank.append((gi, (bb - g0) // 4))
                cursor += nch
            call_plan.append((gi, qq, c0, cursor - c0))
    first_of, last_of = {}, {}
    for ci, bkey in enumerate(chunk_bank):
        if bkey not in first_of:
            first_of[bkey] = ci
        last_of[bkey] = ci
    chunk_meta = [
        (chunk_blk[ci], first_of[chunk_bank[ci]] == ci,
         last_of[chunk_bank[ci]] == ci)
        for ci in range(len(chunk_blk))
    ]
    return seg_off, chunk_meta, call_plan, cursor


def _build_structure(num_nodes, edge_index, z, maxz):
    N = int(num_nodes)
    NSH = N // NCORES
    QROWS = N // NQ
    NBLK = (NSH + P - 1) // P
    GSH = NSH // NPG

    src = np.asarray(edge_index[0], dtype=np.int64)
    dst = np.asarray(edge_index[1], dtype=np.int64)
    loops = np.arange(N, dtype=np.int64)
    src = np.concatenate([src, loops])
    dst = np.concatenate([dst, loops])
    deg = np.bincount(dst, minlength=N).astype(np.float32)
    dinv = (1.0 / np.sqrt(np.maximum(deg, 1.0))).astype(np.float32)
    dinv[deg <= 0] = 0.0
    norm = dinv[src] * dinv[dst]
    zsrc = np.asarray(z, dtype=np.int64)[src]
    MAXZP = ((maxz + P - 1) // P) * P

    # ---- compact destination set S: sources of edges into center nodes ----
    is_c = (dst % NPG) < 2
    S = np.unique(src[is_c])            # sorted; includes centers (self-loops)
    comp = np.full(N, -1, dtype=np.int64)
    Score = S // NSH
    NS = np.bincount(Score, minlength=NCORES)
    NB2 = int(max(1, -(-NS.max() // P)))
    SP2 = NB2 * P
    start_c = np.zeros(NCORES + 1, dtype=np.int64)
    np.cumsum(NS, out=start_c[1:])
    comp[S] = np.arange(len(S)) - start_c[Score]

    # ---- layer-2 sweep: edges with dst in S, compact columns --------------
    m2 = comp[dst] >= 0
    src2, dst2 = src[m2], dst[m2]
    core2 = dst2 // NSH
    q2 = src2 // QROWS
    cpos2 = comp[dst2]
    b2 = cpos2 // P
    dl2 = cpos2 % P
    key2 = (core2 * NQ + q2) * NB2 + b2
    cnt2 = np.bincount(key2, minlength=NCORES * NQ * NB2) \
        .reshape(NCORES, NQ, NB2)
    seg2 = np.maximum((cnt2.max(axis=0) + P - 1) // P, 1)     # [NQ, NB2]
    order2 = np.lexsort((src2, b2, q2, core2))
    src2s, dl2s = src2[order2], dl2[order2]
    groups2 = [list(range(g, min(g + BG, NB2))) for g in range(0, NB2, BG)]
    seg_off2, chunk_meta2, call_plan2, NCHUNK2 = \
        _chunk_layout(seg2, groups2, NQ)
    NSLOT2 = NCHUNK2 * P
    starts2 = np.zeros(NCORES * NQ * NB2 + 1, dtype=np.int64)
    np.cumsum(cnt2.reshape(-1), out=starts2[1:])

    # ---- layer-3 sweep: edges into centers; compact gather table ----------
    src3, dst3 = src[is_c], dst[is_c]
    core3 = dst3 // NSH
    row3 = (src3 // NSH) * SP2 + comp[src3]
    VR3 = (NCORES * SP2) // NQ3
    assert VR3 <= (1 << 15), VR3
    q3 = row3 // VR3
    i3 = row3 % VR3
    cl3 = 2 * ((dst3 % NSH) // NPG) + (dst3 % NPG)            # [0, 2*GSH)
    cnt3 = np.bincount(core3 * NQ3 + q3, minlength=NCORES * NQ3) \
        .reshape(NCORES, NQ3)
    seg3 = np.maximum((cnt3.max(axis=0) + P - 1) // P, 1)     # [NQ3]
    NCHUNK3 = int(seg3.sum())
    NSLOT3 = NCHUNK3 * P
    order3 = np.lexsort((row3, q3, core3))
    i3s, cl3s = i3[order3], cl3[order3]
    starts3 = np.zeros(NCORES * NQ3 + 1, dtype=np.int64)
    np.cumsum(cnt3.reshape(-1), out=starts3[1:])
    seg3_off = np.zeros(NQ3, dtype=np.int64)
    c3 = 0
    for qq in range(NQ3):
        seg3_off[qq] = c3 * P
        c3 += int(seg3[qq])

    per_core = []
    for c in range(NCORES):
        idx2 = np.zeros(NSLOT2, dtype=np.int16)
        dlw2 = np.full(NSLOT2, -1.0, dtype=GDT_NP)
        for qq in range(NQ):
            for bb in range(NB2):
                k = (c * NQ + qq) * NB2 + bb
                s0, s1 = starts2[k], starts2[k + 1]
                n = s1 - s0
                o = seg_off2[qq, bb]
                idx2[o:o + n] = (src2s[s0:s1] % QROWS).astype(np.int16)
                dlw2[o:o + n] = dl2s[s0:s1].astype(GDT_NP)

        idx3 = np.zeros(NSLOT3, dtype=np.int16)
        cl3w = np.full(NSLOT3, -1.0, dtype=GDT_NP)
        for qq in range(NQ3):
            k = c * NQ3 + qq
            s0, s1 = starts3[k], starts3[k + 1]
            n = s1 - s0
            o = seg3_off[qq]
            idx3[o:o + n] = (i3s[s0:s1]).astype(np.int16)
            cl3w[o:o + n] = cl3s[s0:s1].astype(GDT_NP)

        # layer-1 as a dense matmul: C1[z, local_dst] = sum of norms
        NSHP = NBLK * P
        ct1 = np.zeros((MAXZP, NSHP), dtype=np.float32)
        mc = (dst // NSH) == c
        np.add.at(ct1, (zsrc[mc], dst[mc] - c * NSH), norm[mc])

        # dinv tiles
        dloc = np.zeros(NSHP, dtype=np.float32)
        dloc[:NSH] = dinv[c * NSH:(c + 1) * NSH]
        dinv_sh = np.ascontiguousarray(dloc.reshape(NBLK, P).T)
        Sc = S[start_c[c]:start_c[c + 1]]
        dc = np.zeros(SP2, dtype=np.float32)
        dc[:len(Sc)] = dinv[Sc]
        dinv_c2 = np.ascontiguousarray(dc.reshape(NB2, P).T)
        dinvrep2 = np.tile(dc[None, :], (P, 1)).astype(np.float32)
        centers = c * NSH + (np.arange(GSH)[:, None] * NPG
                             + np.array([0, 1])[None, :]).ravel()
        dc3 = np.zeros(2 * P, dtype=np.float32)
        dc3[:2 * GSH] = dinv[centers]
        dinvc3 = np.tile(dc3[None, :], (P, 1)).astype(np.float32)

        per_core.append({
            "idxh": _pack_idx(idx2),
            "dlw": np.ascontiguousarray(dlw2.reshape(NCHUNK2, P).T),
            "idx3": _pack_idx(idx3),
            "cl3w": np.ascontiguousarray(cl3w.reshape(NCHUNK3, P).T),
            "CT1": ct1.astype(GDT_NP),
            "dinv_sh": dinv_sh,
            "dinv_c2": dinv_c2,
            "dinvrep2": dinvrep2,
            "dinvc3": dinvc3,
        })

    struct = {
        "N": N, "NSH": NSH, "QROWS": QROWS, "NBLK": NBLK,
        "NB2": NB2, "SP2": SP2, "VR3": VR3,
        "NCHUNK2": NCHUNK2, "NSLOT2": NSLOT2,
        "seg2": seg2, "groups2": groups2,
        "chunk_meta2": chunk_meta2, "call_plan2": call_plan2,
        "NCHUNK3": NCHUNK3, "NSLOT3": NSLOT3, "seg3": seg3,
        "MAXZP": MAXZP,
    }
    return struct, per_core


# --------------------------------------------------------------------------
# device kernel builder
# --------------------------------------------------------------------------

def _build_kernel(struct, num_graphs, maxz=1000):
    import concourse.bass as bass
    import concourse.tile as tile
    import concourse.mybir as mybir
    from concourse import bacc

    f32 = mybir.dt.float32
    f16 = mybir.dt.float16 if GDT_NP == np.float16 else mybir.dt.bfloat16
    i16 = mybir.dt.int16
    RELU = mybir.ActivationFunctionType.Relu
    COPY = mybir.ActivationFunctionType.Identity

    N, NSH, QROWS = struct["N"], struct["NSH"], struct["QROWS"]
    NBLK = struct["NBLK"]
    NB2, SP2, VR3 = struct["NB2"], struct["SP2"], struct["VR3"]
    NCHUNK2, NCHUNK3 = struct["NCHUNK2"], struct["NCHUNK3"]
    groups2 = struct["groups2"]
    chunk_meta2 = struct["chunk_meta2"]
    call_plan2 = struct["call_plan2"]
    seg3 = struct["seg3"]
    NSHP = NBLK * P
    GSH = NSH // NPG
    W3C = 2 * GSH                   # compact layer-3 columns (250)
    MAXZP = struct["MAXZP"]
    NZC = MAXZP // P

    nc = bacc.Bacc("TRN2", target_bir_lowering=False, debug=False,
                   num_devices=NCORES)

    # ---- I/O
    idxh_d = nc.dram_tensor("idxh", [P, struct["NSLOT2"] // 16], i16,
                            kind="ExternalInput")
    ct1_d = nc.dram_tensor("CT1", [MAXZP, NSHP], f16, kind="ExternalInput")
    t1z_d = nc.dram_tensor("T1Z", [P, MAXZP], f16, kind="ExternalInput")
    dl_d = nc.dram_tensor("dlw", [P, NCHUNK2], f16, kind="ExternalInput")
    idx3_d = nc.dram_tensor("idx3", [P, struct["NSLOT3"] // 16], i16,
                            kind="ExternalInput")
    cl3_d = nc.dram_tensor("cl3w", [P, NCHUNK3], f16, kind="ExternalInput")
    dinv_sh_d = nc.dram_tensor("dinv_sh", [P, NBLK], f32, kind="ExternalInput")
    dinv_c2_d = nc.dram_tensor("dinv_c2", [P, NB2], f32, kind="ExternalInput")
    dinvrep2_d = nc.dram_tensor("dinvrep2", [P, SP2], f32, kind="ExternalInput")
    dinvc3_d = nc.dram_tensor("dinvc3", [P, 2 * P], f32, kind="ExternalInput")
    W_d = [nc.dram_tensor(f"W{i}", [P, P], f32, kind="ExternalInput")
           for i in (1, 2, 3)]
    b_d = [nc.dram_tensor(f"b{i}", [P, 1], f32, kind="ExternalInput")
           for i in (1, 2, 3)]
    mw1_d = nc.dram_tensor("mw1", [P, P], f32, kind="ExternalInput")
    mw2_d = nc.dram_tensor("mw2", [P, 1], f32, kind="ExternalInput")
    mb1_d = nc.dram_tensor("mb1", [P, 1], f32, kind="ExternalInput")
    mb2_d = nc.dram_tensor("mb2", [1, 1], f32, kind="ExternalInput")
    y_d = nc.dram_tensor("y", [1, GSH], f32, kind="ExternalOutput")

    with tile.TileContext(nc) as tc, ExitStack() as ctx:
        dram = ctx.enter_context(tc.tile_pool(name="dram", bufs=1, space="DRAM"))
        const = ctx.enter_context(tc.tile_pool(name="const", bufs=1))
        work = ctx.enter_context(tc.tile_pool(name="work", bufs=4))
        msgp = ctx.enter_context(tc.tile_pool(name="msgp", bufs=3))
        ohp = ctx.enter_context(tc.tile_pool(name="ohp", bufs=3))
        stage_p = ctx.enter_context(tc.tile_pool(name="stagep", bufs=2))
        ps_sc = ctx.enter_context(tc.tile_pool(name="ps_sc", bufs=2, space="PSUM"))
        ps_mm = ctx.enter_context(tc.tile_pool(name="ps_mm", bufs=2, space="PSUM"))
        ps_l3 = ctx.enter_context(tc.tile_pool(name="ps_l3", bufs=1, space="PSUM"))

        hsh = dram.tile([NSHP, H], f16)
        hsh3 = dram.tile([SP2, H], f16)

        # ---- constants
        iota_i = const.tile([P, 2 * P], mybir.dt.int32)
        nc.gpsimd.iota(iota_i[:], pattern=[[1, 2 * P]], base=0,
                       channel_multiplier=0)
        iota_h = const.tile([P, 2 * P], f16)
        nc.vector.tensor_copy(iota_h[:], iota_i[:])

        dl_t = const.tile([P, NCHUNK2], f16)
        nc.sync.dma_start(dl_t[:], dl_d[:])
        cl3_t = const.tile([P, NCHUNK3], f16)
        nc.sync.dma_start(cl3_t[:], cl3_d[:])
        t1z_t = const.tile([P, NZC, P], f16)
        nc.sync.dma_start(t1z_t[:], t1z_d[:].rearrange("p (z f) -> p z f", f=P))
        dinv_sh_t = const.tile([P, NBLK], f32)
        nc.sync.dma_start(dinv_sh_t[:], dinv_sh_d[:])
        dinv_c2_t = const.tile([P, NB2], f32)
        nc.sync.dma_start(dinv_c2_t[:], dinv_c2_d[:])
        dinvrep2_t = const.tile([P, SP2], f32)
        nc.sync.dma_start(dinvrep2_t[:], dinvrep2_d[:])
        dinvc3_t = const.tile([P, 2 * P], f32)
        nc.sync.dma_start(dinvc3_t[:], dinvc3_d[:])
        W_t, b_t = [], []
        for i in range(3):
            wf = const.tile([P, P], f32, name=f"wf{i}")
            nc.sync.dma_start(wf[:], W_d[i][:])
            w = const.tile([P, P], f16, name=f"w{i}")
            nc.vector.tensor_copy(w[:], wf[:])
            W_t.append(w)
            b = const.tile([P, 1], f32, name=f"bt{i}")
            nc.sync.dma_start(b[:], b_d[i][:])
            b_t.append(b)
        mw1_t = const.tile([P, P], f32)
        nc.sync.dma_start(mw1_t[:], mw1_d[:])
        mw2_t = const.tile([P, 1], f32)
        nc.sync.dma_start(mw2_t[:], mw2_d[:])
        mb1_t = const.tile([P, 1], f32)
        nc.sync.dma_start(mb1_t[:], mb1_d[:])
        mb2_t = const.tile([1, 1], f32)
        nc.sync.dma_start(mb2_t[:], mb2_d[:])

        xA = const.tile([P, NSHP], f16)      # layer-1 output [feat, dst]
        xB = const.tile([P, SP2], f16)       # layer-2 output (compact)

        # resident gather indices
        idxh_t = const.tile([P, struct["NSLOT2"] // 16], i16)
        nc.sync.dma_start(idxh_t[:], idxh_d[:])
        idx3_t = const.tile([P, struct["NSLOT3"] // 16], i16)
        nc.sync.dma_start(idx3_t[:], idx3_d[:])

        # msg pool buffers start as garbage SBUF; memset so any slot the
        # matmul touches before a gather lands holds finite data
        MSGCH = max(GG, int(np.asarray(seg3).max()))
        for _i in range(3):
            m0 = msgp.tile([P, MSGCH, H], f16, tag="msg")
            nc.vector.memset(m0[:], 0.0)

        # ---- layer 1 as dense matmul: xA = relu(T1^T @ C1 + b1)
        def l1_matmul():
            DC = 512
            for d0 in range(0, NSHP, DC):
                dw = min(DC, NSHP - d0)
                ps = ps_mm.tile([P, DC], f32, tag="mm")
                for zc in range(NZC):
                    ct = work.tile([P, DC], f16, tag="ct")
                    nc.sync.dma_start(ct[:, :dw],
                                      ct1_d[zc * P:(zc + 1) * P, d0:d0 + dw])
                    nc.tensor.matmul(ps[:, :dw], lhsT=t1z_t[:, zc, :],
                                     rhs=ct[:, :dw],
                                     start=(zc == 0), stop=(zc == NZC - 1))
                nc.scalar.activation(out=xA[:, d0:d0 + dw], in_=ps[:, :dw],
                                     func=RELU, bias=b_t[0][:], scale=1.0)

        # ---- h phase: hsh[r] = dinv[r] * (x @ W)[r] -> AllGather
        def h_phase(xin, w_t, dinv_ap, nblk, rows, hsh_t, hfull_t):
            for r0 in range(0, nblk, 4):
                jn = min(4, nblk - r0)
                st = stage_p.tile([P, 4, H], f16, tag="hst")
                for j in range(jn):
                    r = r0 + j
                    m = min(P, rows - r * P)
                    ps = ps_mm.tile([P, P], f32, tag="mm")
                    nc.tensor.matmul(ps[:m, :], lhsT=xin[:, r * P:r * P + m],
                                     rhs=w_t[:], start=True, stop=True)
                    nc.scalar.activation(out=st[:, j, :], in_=ps[:, :],
                                         func=COPY, bias=0.0,
                                         scale=dinv_ap[:, r:r + 1])
                nc.sync.dma_start(
                    hsh_t[r0 * P:(r0 + jn) * P, :]
                        .rearrange("(j p) f -> p j f", p=P),
                    st[:, :jn, :])
            nc.gpsimd.collective_compute(
                "AllGather", mybir.AluOpType.bypass,
                replica_groups=[list(range(NCORES))],
                ins=[hsh_t[:rows, :].opt()],
                outs=[hfull_t[:].opt()])

        # ---- layer-2 scatter sweep into compact columns
        def scatter_sweep(table_views):
            for gi, blocks in enumerate(groups2):
                g0 = blocks[0]
                gw = len(blocks)
                psg = ps_sc.tile([P, gw * P], f32, tag="sc")
                for qq in range(NQ):
                    _, _, c0, nch = call_plan2[gi * NQ + qq]
                    s = 0
                    while s < nch:
                        g = min(GG, nch - s)
                        cc0 = c0 + s
                        nidx = g * P
                        msg = msgp.tile([P, g, H], f16, tag="msg")
                        nc.gpsimd.dma_gather(
                            msg[:], table_views[qq],
                            idxh_t[:, cc0 * 8:(cc0 + g) * 8], nidx, nidx, H,
                            single_packet=False)
                        ohc = ohp.tile([P, g, P], f16, tag="oh")
                        nc.vector.tensor_tensor(
                            out=ohc[:],
                            in0=iota_h[:, None, :P].to_broadcast([P, g, P]),
                            in1=dl_t[:, cc0:cc0 + g][:, :, None]
                                .to_broadcast([P, g, P]),
                            op=mybir.AluOpType.is_equal)
                        for j in range(g):
                            bb, first, last = chunk_meta2[cc0 + j]
                            col = (bb - g0) * P
                            nc.tensor.matmul(
                                psg[:, col:col + P], lhsT=msg[:, j, :],
                                rhs=ohc[:, j, :], start=first, stop=last)
                        s += g
                # flush group: dst-side dinv scale, then bias + relu
                tmp = work.tile([P, gw * P], f32, tag="fl")
                nc.vector.tensor_tensor(
                    out=tmp[:], in0=psg[:],
                    in1=dinvrep2_t[:, g0 * P:(g0 + gw) * P],
                    op=mybir.AluOpType.mult)
                for bb in blocks:
                    col = (bb - g0) * P
                    nc.scalar.activation(
                        out=xB[:, bb * P:(bb + 1) * P],
                        in_=tmp[:, col:col + P],
                        func=RELU, bias=b_t[1][:], scale=1.0)

        # ---- layer-3 compact sweep: one [P, 256] PSUM over center columns
        def sweep_l3(table_views):
            ps3 = ps_l3.tile([P, 2 * P], f32, tag="l3")
            cursor = 0
            for qq in range(NQ3):
                nch = int(seg3[qq])
                s = 0
                while s < nch:
                    g = min(GG, nch - s)
                    cc0 = cursor + s
                    nidx = g * P
                    msg = msgp.tile([P, g, H], f16, tag="msg")
                    nc.gpsimd.dma_gather(
                        msg[:], table_views[qq],
                        idx3_t[:, cc0 * 8:(cc0 + g) * 8], nidx, nidx, H,
                        single_packet=False)
                    oh = ohp.tile([P, g, 2 * P], f16, tag="oh3")
                    nc.vector.tensor_tensor(
                        out=oh[:],
                        in0=iota_h[:, None, :].to_broadcast([P, g, 2 * P]),
                        in1=cl3_t[:, cc0:cc0 + g][:, :, None]
                            .to_broadcast([P, g, 2 * P]),
                        op=mybir.AluOpType.is_equal)
                    for j in range(g):
                        ci = cc0 + j
                        nc.tensor.matmul(
                            ps3[:], lhsT=msg[:, j, :], rhs=oh[:, j, :],
                            start=(ci == 0), stop=(ci == NCHUNK3 - 1))
                    s += g
                cursor += nch
            tmp3 = work.tile([P, 2 * P], f32, tag="fl3")
            nc.vector.tensor_tensor(out=tmp3[:], in0=ps3[:], in1=dinvc3_t[:],
                                    op=mybir.AluOpType.mult)
            x3c = const.tile([P, 2 * P], f32)
            nc.scalar.activation(out=x3c[:], in_=tmp3[:], func=COPY,
                                 bias=b_t[2][:], scale=1.0)
            return x3c

        # ---- pipeline
        hfull2 = dram.tile([N, H], f16, addr_space="Shared", name="hfull2")
        hfull3 = dram.tile([NCORES * SP2, H], f16, addr_space="Shared",
                           name="hfull3")

        l1_matmul()
        h_phase(xA, W_t[1], dinv_sh_t, NBLK, NSH, hsh, hfull2)
        h2_views = [hfull2[qq * QROWS:(qq + 1) * QROWS, :] for qq in range(NQ)]
        scatter_sweep(h2_views)
        h_phase(xB, W_t[2], dinv_c2_t, NB2, SP2, hsh3, hfull3)
        h3_views = [hfull3[qq * VR3:(qq + 1) * VR3, :] for qq in range(NQ3)]
        x3c = sweep_l3(h3_views)

        # ---- readout: p = x3[2g] * x3[2g+1]; y = relu(p@mw1+mb1)@mw2+mb2
        x3r = x3c[:, :W3C].rearrange("p (g r) -> p g r", r=2)
        pT = const.tile([P, GSH], f32)
        nc.vector.tensor_tensor(
            out=pT[:], in0=x3r[:, :, 0], in1=x3r[:, :, 1],
            op=mybir.AluOpType.mult)
        hps = ps_mm.tile([P, GSH], f32, tag="mm")
        nc.tensor.matmul(hps[:], lhsT=mw1_t[:], rhs=pT[:], start=True, stop=True)
        hT = const.tile([P, GSH], f32)
        nc.scalar.activation(out=hT[:], in_=hps[:], func=RELU,
                             bias=mb1_t[:], scale=1.0)
        yps = ps_mm.tile([1, GSH], f32, tag="mm")
        nc.tensor.matmul(yps[:], lhsT=mw2_t[:], rhs=hT[:], start=True, stop=True)
        ysb = const.tile([1, GSH], f32)
        nc.scalar.activation(out=ysb[:], in_=yps[:], func=COPY,
                             bias=mb2_t[:], scale=1.0)
        nc.sync.dma_start(y_d[:], ysb[:])

    nc.compile()
    return nc


# --------------------------------------------------------------------------
# entry point
# --------------------------------------------------------------------------

def _make_in_maps(inputs, per_core):
    z_table = np.asarray(inputs["z_table"], np.float32)
    W1 = np.asarray(inputs["W1"], np.float32)
    maxz = z_table.shape[0]
    MAXZP = ((maxz + P - 1) // P) * P
    NZC = MAXZP // P
    t1 = np.zeros((MAXZP, H), np.float32)
    t1[:maxz] = z_table @ W1
    t1z = np.ascontiguousarray(
        t1.reshape(NZC, P, H).transpose(1, 0, 2).reshape(P, MAXZP)
    ).astype(GDT_NP)
    common = {
        "T1Z": t1z,
        "W1": W1, "W2": np.asarray(inputs["W2"], np.float32),
        "W3": np.asarray(inputs["W3"], np.float32),
        "b1": np.asarray(inputs["b1"], np.float32).reshape(P, 1),
        "b2": np.asarray(inputs["b2"], np.float32).reshape(P, 1),
        "b3": np.asarray(inputs["b3"], np.float32).reshape(P, 1),
        "mw1": np.asarray(inputs["mw1"], np.float32),
        "mw2": np.asarray(inputs["mw2"], np.float32).reshape(P, 1),
        "mb1": np.asarray(inputs["mb1"], np.float32).reshape(P, 1),
        "mb2": np.asarray(inputs["mb2"], np.float32).reshape(1, 1),
    }
    return [dict(common, **per_core[c]) for c in range(NCORES)]


def kernel(num_nodes, z, edge_index, batch, num_graphs,
           z_table, W1, b1, W2, b2, W3, b3, mw1, mb1, mw2, mb2,
           _want_results=False):
    from concourse.bass_utils import run_bass_kernel_spmd

    num_nodes = int(num_nodes)
    num_graphs = int(num_graphs)
    z = np.asarray(z)
    edge_index = np.asarray(edge_index)

    struct, per_core = _build_structure(num_nodes, edge_index, z,
                                        np.asarray(z_table).shape[0])
    nc = _build_kernel(struct, num_graphs, maxz=np.asarray(z_table).shape[0])

    inputs = {"z_table": z_table, "W1": W1, "W2": W2, "W3": W3,
              "b1": b1, "b2": b2, "b3": b3, "mw1": mw1, "mw2": mw2,
              "mb1": mb1, "mb2": mb2}
    in_maps = _make_in_maps(inputs, per_core)

    res = run_bass_kernel_spmd(nc, in_maps, core_ids=list(range(NCORES)),
                               trace=bool(int(__import__("os").environ.get(
                                   "GCN_TRACE", "0"))))
    ys = [res.results[c]["y"].reshape(-1, 1) for c in range(NCORES)]
    out = np.concatenate(ys, 0).astype(np.float32)
    if _want_results:
        return out, res
    return out
